# revision 10
# baseline (speedup 1.0000x reference)
"""Trainium2 Bass kernel for DistillLossSimpleMSE (segment_reduce).

Math (per object o, with uniform segments of P points):
    x   = net_out[o*P:(o+1)*P]                [P, D]
    m   = mask_pts[o]                         [M, P] in {0,1}
    e   = nan_to_num(mask_embs[o*M:(o+1)*M])  [M, D]
    sum_sq = sum_m [ sum_p m*||x_p||^2 + cnt_m*||e_m||^2 - 2 e_m . (sum_p m x_p) ]
    out = sum_sq / (D * total_points)

Sharding: object-parallel, 1 object per core (8 objects, 8 cores).

Device kernel per core accumulates in PSUM over all P points with SWAPPED
matmul operands: x (and x*x) is the STATIONARY operand -- exactly 128 bf16
columns, so the compiler's Fast Weight Load (4 cols/cycle) kicks in -- and the
32 mask columns are the MOVING operand.  Per 128-point class that is two
matmuls of ~40 ns each instead of streaming 256 x-columns, so the PE fits the
DMA window even at the cold (1.2 GHz) HAM clock.

    ps8[d, 2q+0, m] += sum_p x[p, d]   * mask[m, p]     (points of quarter q)
    ps8[d, 2q+1, m] += sum_p x[p, d]^2 * mask[m, p]

All 1024 matmuls accumulate into ONE 2 KiB PSUM bank: the first matmul
(start=True) clears the bank's has_written bits, every later slice's first
write lands as overwrite-where-clear, then accumulates.  mx[m, d] and
t1[m] = sum_d,q ps8[d, 2q+1, m] are finished on the host in f64.
cnt comes from the mask-convert pass (DVE accum).

DMA-boundness: per core 40 MB (32 MB x + 8 MB mask) over 16 DMA engines at
~25.6 B/ns each => ~98 us floor.  All DMA triggers are HWDGE (never
nc.gpsimd.dma_start: SWDGE descriptor generation shares the DVE/GpSimd SBUF
port pair and starves whenever DVE runs a 2-port op).  The mask lands as
whole 2 MB blocks in 3 dedicated rotating buffers whose triggers go out on
the Sync queue interleaved with the first x tiles and with no WAR waits, so
no trigger ever head-of-line blocks the x stream; ident/cnts ride the Scalar
queue.  The mask-block int32->bf16 convert is one whole-block DVE op issued a
block ahead of its transposes.

Layout trick for full DMA bandwidth: a straight contiguous [128, 4096] tile of
x (16 KB/partition descriptors) has partition p holding rows 32p..32p+31, so
its column-slice [:, c*128:(c+1)*128] is exactly the [point, d] operand for
the stride-32 point class {base + 32p + c}.  The mask (contiguous [128,
16384] int32 view) is transposed on-chip through the PE with a stride-32 free
AP so its chunks cover the same point classes.

The last x tile is fetched and processed as 4 separate 1024-col chunks so the
end-of-stream serial chain (convert -> square -> matmuls -> out) is a quarter
as long.

Multi-wait instructions are legalized via bass_rust.generate_event_semaphores
(TRN2 allows only one semaphore wait per compute instruction).
"""

import os

import numpy as np
import ml_dtypes

import bass_rust
import concourse.bass as bass
import concourse.mybir as mybir
import concourse.tile as tile
from concourse.bass_utils import run_bass_kernel_spmd

N_CORES = 8
N_OBJ, P, M, D = 8, 65536, 32, 128

VIEW_P = 128                 # mask flat view partitions
VIEW_F = M * P // VIEW_P     # 16384 view cols; view[r, f] = mask[r//4, (r%4)*16384 + f]
BLK = 4096                   # view cols per block (= points per x-tile)
NBLK = VIEW_F // BLK         # 4 mask blocks
NCLS = BLK // 128            # 32 stride-32 point classes per block
NT = 16                      # x tiles of [128, 4096]
OUTW = 8 * M                 # 256 dram out cols: [d, 2q+t, m] flattened
NXB = 4                      # x-tile landing buffers (f32)
NRHS = 3                     # [x | x^2] bf16 stationary-operand buffers
NMI = 3                      # mask block landing buffers (int32)
TAILC = 4                    # last x tile split into this many chunks

F32 = mybir.dt.float32
BF16 = mybir.dt.bfloat16
I32 = mybir.dt.int32

LAST = None      # BassKernelResults of the most recent run (for test harness)
_NC_CACHE = {}


def _build_nc():
    nc = bass.Bass()
    x = nc.dram_tensor("x", [P, D], F32, kind="ExternalInput")
    mask = nc.dram_tensor("mask", [VIEW_P, VIEW_F], I32, kind="ExternalInput")
    out = nc.dram_tensor("out", [D, OUTW], F32, kind="ExternalOutput")
    # per-(mask,quarter)-row, per-block partial point counts (see host finale)
    cnts = nc.dram_tensor("cnts", [VIEW_P, NBLK], F32, kind="ExternalOutput")

    # x tile view: [16 tiles, 128 partitions, 32*128 contiguous]
    xt = x[:, :].rearrange("(j p s) d -> j p (s d)", p=128, s=BLK // 128)

    with tile.TileContext(nc) as tc:
        with (
            tc.tile_pool(name="singles", bufs=1) as singles,
            tc.tile_pool(name="psingles", bufs=1, space="PSUM") as psingles,
        ):
            # Persistent tiles only: pool-reallocated tiles go through Tile's
            # release machinery whose extra waits collide with the PE 1-wait
            # codegen limit more often.
            ident_const = nc.inline_tensor(
                np.eye(128, dtype=np.float32).astype(ml_dtypes.bfloat16),
                name="identc",
            )
            ident = singles.tile([128, 128], BF16, tag="ident")
            nc.scalar.dma_start(out=ident, in_=ident_const[:, :])

            # stationary-operand buffers: per-class contiguous [x | x^2]
            # 128-col halves (FWL needs a contiguous 128-col weight load)
            rhs_bufs = [
                singles.tile(
                    [128, NCLS, 2 * D], BF16, name=f"rhsbuf{j}", tag=f"rhsbuf{j}"
                )
                for j in range(NRHS)
            ]
            cnt_sb = singles.tile([VIEW_P, NBLK], F32, tag="cnt_sb")

            xb_bufs = [
                singles.tile([128, BLK], F32, name=f"xb{j}", tag=f"xb{j}")
                for j in range(NXB)
            ]
            mi_bufs = [
                singles.tile([VIEW_P, BLK], I32, name=f"mi{j}", tag=f"mi{j}")
                for j in range(NMI)
            ]
            mf_bufs = [
                singles.tile([VIEW_P, BLK], BF16, name=f"mf{j}", tag=f"mf{j}")
                for j in range(2)
            ]
            # mt written once per block, read only during that block's matmuls
            mt_bufs = [
                singles.tile([VIEW_P, BLK], BF16, name=f"mt{j}", tag=f"mt{j}")
                for j in range(3)
            ]
            ps4_bufs = [
                psingles.tile([128, 8, 128], BF16, name=f"ps4{j}", tag=f"ps4{j}")
                for j in range(2)
            ]
            # One full 2 KiB PSUM bank: 8 slices [d, 2q+t, 0:M] (t=0 -> m@x,
            # t=1 -> m@x^2); cols M..64 of each slice are dead padding.
            ps8 = psingles.tile([128, 8, 64], F32, tag="ps8")

            n_mm = NBLK * 4 * NCLS * 2

            def mask_dma(b):
                nc.sync.dma_start(
                    out=mi_bufs[b % NMI], in_=mask[:, b * BLK:(b + 1) * BLK]
                )

            def mask_stt(b):
                # whole-block int32->bf16 convert + free-dim count, one DVE op
                # (measured: DVE STT 4.4 us/block vs 5.6 us for an ACT copy)
                nc.vector.scalar_tensor_tensor(
                    out=mf_bufs[b % 2],
                    in0=mi_bufs[b % NMI], scalar=0.0, in1=mi_bufs[b % NMI],
                    op0=mybir.AluOpType.add,
                    op1=mybir.AluOpType.bypass,
                    accum_out=cnt_sb[:, b:b + 1],
                )

            def transposes(b, h0, h1):
                # Transpose the mask block through the PE with stride-32 free
                # APs: transpose c yields, for every quarter q, the moving-
                # operand mask columns of point class {q*16384 + b*4096 +
                # 32p + c}.
                mf = mf_bufs[b % 2]
                mt = mt_bufs[b % 3]
                # f' = 32p + c: class c picks stride-32 free elems
                mfv = mf.rearrange("r (p c) -> r c p", c=NCLS)
                for h in range(h0, h1):
                    ps4 = ps4_bufs[h % 2]
                    for tt in range(8):
                        c = h * 8 + tt
                        nc.tensor.transpose(ps4[:, tt, :], mfv[:, c, :], ident)
                    nc.scalar.copy(
                        mt[:, h * 1024:(h + 1) * 1024],
                        ps4.rearrange("p t d -> p (t d)"),
                    )

            def x_chunk(jx, xb, rhs, c0, c1):
                # Per-chunk stages for classes [c0, c1): DVE converts the f32
                # landing slice to the bf16 x half (2-port 2x mode -- safe,
                # nothing here uses the GpSimd shared port); the square into
                # the x^2 half runs on ACT mid-stream (it has slack there and
                # keeping DVE short keeps the DMA-pacing conv chain tight),
                # but on DVE for the last block, where ACT's ~0.9 ns/elem
                # squares would otherwise serialize the whole tail.
                xbv = xb.rearrange("p (s d) -> p s d", s=NCLS)
                nc.vector.tensor_scalar_mul(
                    rhs[:, c0:c1, 0:D], xbv[:, c0:c1, :], 1.0
                )
                if jx < NT - 4:
                    nc.scalar.square(
                        rhs[:, c0:c1, D:2 * D], rhs[:, c0:c1, 0:D]
                    )
                else:
                    nc.vector.tensor_mul(
                        rhs[:, c0:c1, D:2 * D],
                        rhs[:, c0:c1, 0:D],
                        rhs[:, c0:c1, 0:D],
                    )

            k = 0
            jx = 0
            mask_dma(0)
            mask_stt(0)
            transposes(0, 0, NCLS // 8)
            for b in range(NBLK):
                mtv = mt_bufs[b % 3].rearrange(
                    "p (c m q) -> p c q m", c=NCLS, m=M, q=4
                )
                for q in range(4):
                    if b == 0 and q >= 1:
                        mask_dma(q)       # all mask triggers early on Sync
                    j = q * NBLK + b       # x tile covering this block+quarter
                    xb = xb_bufs[jx % NXB]
                    rhs = rhs_bufs[jx % NRHS]
                    last_tile = jx == NT - 1
                    jx += 1
                    nchunk = TAILC if last_tile else 1
                    ccls = NCLS // nchunk
                    for ch in range(nchunk):
                        c0, c1 = ch * ccls, (ch + 1) * ccls
                        nc.sync.dma_start(
                            out=xb[:, c0 * D:c1 * D],
                            in_=xt[j, :, c0 * D:c1 * D],
                        )
                        x_chunk(jx - 1, xb, rhs, c0, c1)
                        for c in range(c0, c1):
                            for t in range(2):
                                nc.tensor.matmul(
                                    ps8[:, 2 * q + t, 0:M],
                                    lhsT=rhs[:, c, t * D:(t + 1) * D],
                                    rhs=mtv[:, c, q, :],
                                    start=(k == 0),
                                    stop=(k == n_mm - 1),
                                    skip_group_check=True,
                                )
                                k += 1
                    # Software pipeline: next block's mask convert/transpose
                    # interleaves with this block's matmul stream.
                    if b + 1 < NBLK:
                        if q == 1:
                            mask_stt(b + 1)
                        if q >= 2:
                            h0 = (q - 2) * 2
                            transposes(b + 1, h0, h0 + 2)

            outs = singles.tile([128, OUTW], F32, tag="outs")
            nc.vector.tensor_copy(
                outs.rearrange("p (i m) -> p i m", i=8), ps8[:, :, 0:M]
            )
            nc.sync.dma_start(out=out[:, :], in_=outs)
            nc.scalar.dma_start(out=cnts[:, :], in_=cnt_sb)
    # Split multi-wait instructions into EventSemaphore + instruction to
    # satisfy the TRN2 1-wait-per-instruction codegen limit.
    bass_rust.generate_event_semaphores(nc)
    return nc


def _get_nc():
    if "nc" not in _NC_CACHE:
        _NC_CACHE["nc"] = _build_nc()
    return _NC_CACHE["nc"]


def kernel(net_out, pt_offset, mask_embs, mask_pts, logit_scale):
    global LAST
    net_out = np.ascontiguousarray(np.asarray(net_out, dtype=np.float32))
    mask_pts = np.ascontiguousarray(np.asarray(mask_pts, dtype=np.int32))
    mask_embs = np.asarray(mask_embs, dtype=np.float32)

    nc = _get_nc()
    in_maps = [
        {
            "x": net_out[o * P:(o + 1) * P],
            "mask": mask_pts[o].reshape(VIEW_P, VIEW_F),
        }
        for o in range(N_CORES)
    ]
    trace = os.environ.get("KBENCH_TRACE", "0") == "1"
    res = run_bass_kernel_spmd(nc, in_maps, list(range(N_CORES)), trace=trace)
    LAST = res

    # out[d, 2q+t, m]: t=0 -> sum_p x*mask, t=1 -> sum_p x^2*mask (quarter q)
    accs = np.stack([np.asarray(res.results[o]["out"]) for o in range(N_CORES)])
    accs = accs.reshape(N_CORES, D, 4, 2, M).astype(np.float64)
    mx = accs[:, :, :, 0, :].sum(2).transpose(0, 2, 1)   # [8, 32, 128]
    t1 = accs[:, :, :, 1, :].sum((1, 2))                 # [8, 32]
    # cnt[m] = sum over quarters q and blocks of the per-row partials
    cnts = np.stack([np.asarray(res.results[o]["cnts"]) for o in range(N_CORES)])
    cnt = cnts.sum(-1).reshape(N_CORES, M, 4).sum(-1)  # [8, 32]

    emb = np.nan_to_num(
        mask_embs.reshape(N_OBJ, M, D).astype(np.float64),
        nan=0.0, posinf=0.0, neginf=0.0,
    )
    t2 = cnt * (emb * emb).sum(-1)
    t3 = 2.0 * (emb * mx).sum(-1)
    sum_sq = (t1 + t2 - t3).sum()
    total = cnt.sum()
    val = sum_sq / (D * total) if total > 0 else 0.0
    return np.float32(val)


# revision 11
# speedup vs baseline: 1.0303x; 1.0303x over previous
"""Trainium2 Bass kernel for DistillLossSimpleMSE (segment_reduce).

Math (per object o, with uniform segments of P points):
    x   = net_out[o*P:(o+1)*P]                [P, D]
    m   = mask_pts[o]                         [M, P] in {0,1}
    e   = nan_to_num(mask_embs[o*M:(o+1)*M])  [M, D]
    sum_sq = sum_m [ sum_p m*||x_p||^2 + cnt_m*||e_m||^2 - 2 e_m . (sum_p m x_p) ]
    out = sum_sq / (D * total_points)

Sharding: object-parallel, 1 object per core (8 objects, 8 cores).

Device kernel per core accumulates in PSUM over all P points with SWAPPED
matmul operands: x (and x*x) is the STATIONARY operand -- exactly 128 bf16
columns, so the compiler's Fast Weight Load (4 cols/cycle) kicks in -- and the
32 mask columns are the MOVING operand.  Per 128-point class that is two
matmuls of ~40 ns each instead of streaming 256 x-columns, so the PE fits the
DMA window even at the cold (1.2 GHz) HAM clock.

    ps8[d, 2q+0, m] += sum_p x[p, d]   * mask[m, p]     (points of quarter q)
    ps8[d, 2q+1, m] += sum_p x[p, d]^2 * mask[m, p]

All 1024 matmuls accumulate into ONE 2 KiB PSUM bank: the first matmul
(start=True) clears the bank's has_written bits, every later slice's first
write lands as overwrite-where-clear, then accumulates.  mx[m, d] and
t1[m] = sum_d,q ps8[d, 2q+1, m] are finished on the host in f64.
cnt comes from the mask-convert pass (DVE accum).

DMA-boundness: per core 40 MB (32 MB x + 8 MB mask) over 16 DMA engines at
~25.6 B/ns each => ~98 us floor.  All DMA triggers are HWDGE (never
nc.gpsimd.dma_start: SWDGE descriptor generation shares the DVE/GpSimd SBUF
port pair and starves whenever DVE runs a 2-port op).  The mask lands as
whole 2 MB blocks in 3 dedicated rotating buffers whose triggers go out on
the Sync queue interleaved with the first x tiles and with no WAR waits, so
no trigger ever head-of-line blocks the x stream; ident/cnts ride the Scalar
queue.  The mask-block int32->bf16 convert is one whole-block DVE op issued a
block ahead of its transposes.

Layout trick for full DMA bandwidth: a straight contiguous [128, 4096] tile of
x (16 KB/partition descriptors) has partition p holding rows 32p..32p+31, so
its column-slice [:, c*128:(c+1)*128] is exactly the [point, d] operand for
the stride-32 point class {base + 32p + c}.  The mask (contiguous [128,
16384] int32 view) is transposed on-chip through the PE with a stride-32 free
AP so its chunks cover the same point classes.

The last x tile is fetched and processed as 4 separate 1024-col chunks so the
end-of-stream serial chain (convert -> square -> matmuls -> out) is a quarter
as long.

Multi-wait instructions are legalized via bass_rust.generate_event_semaphores
(TRN2 allows only one semaphore wait per compute instruction).
"""

import os

import numpy as np
import ml_dtypes

import bass_rust
import concourse.bass as bass
import concourse.mybir as mybir
import concourse.tile as tile
from concourse.bass_utils import run_bass_kernel_spmd

N_CORES = 8
N_OBJ, P, M, D = 8, 65536, 32, 128

VIEW_P = 128                 # mask flat view partitions
VIEW_F = M * P // VIEW_P     # 16384 view cols; view[r, f] = mask[r//4, (r%4)*16384 + f]
BLK = 4096                   # view cols per block (= points per x-tile)
NBLK = VIEW_F // BLK         # 4 mask blocks
NCLS = BLK // 128            # 32 stride-32 point classes per block
NT = 16                      # x tiles of [128, 4096]
OUTW = 8 * M                 # 256 dram out cols: [d, 2q+t, m] flattened
NXB = 5                      # x-tile landing buffers (f32)
NRHS = 2                     # [x | x^2] bf16 stationary-operand buffers
NMI = 3                      # mask block landing buffers (int32)
TAILC = 4                    # last x tile split into this many chunks

F32 = mybir.dt.float32
BF16 = mybir.dt.bfloat16
I32 = mybir.dt.int32

LAST = None      # BassKernelResults of the most recent run (for test harness)
_NC_CACHE = {}


def _build_nc():
    nc = bass.Bass()
    x = nc.dram_tensor("x", [P, D], F32, kind="ExternalInput")
    mask = nc.dram_tensor("mask", [VIEW_P, VIEW_F], I32, kind="ExternalInput")
    out = nc.dram_tensor("out", [D, OUTW], F32, kind="ExternalOutput")
    # per-(mask,quarter)-row, per-block partial point counts (see host finale)
    cnts = nc.dram_tensor("cnts", [VIEW_P, NBLK], F32, kind="ExternalOutput")

    # x tile view: [16 tiles, 128 partitions, 32*128 contiguous]
    xt = x[:, :].rearrange("(j p s) d -> j p (s d)", p=128, s=BLK // 128)

    with tile.TileContext(nc) as tc:
        with (
            tc.tile_pool(name="singles", bufs=1) as singles,
            tc.tile_pool(name="psingles", bufs=1, space="PSUM") as psingles,
        ):
            # Persistent tiles only: pool-reallocated tiles go through Tile's
            # release machinery whose extra waits collide with the PE 1-wait
            # codegen limit more often.
            ident_const = nc.inline_tensor(
                np.eye(128, dtype=np.float32).astype(ml_dtypes.bfloat16),
                name="identc",
            )
            ident = singles.tile([128, 128], BF16, tag="ident")
            nc.scalar.dma_start(out=ident, in_=ident_const[:, :])

            # stationary-operand buffers: per-class contiguous [x | x^2]
            # 128-col halves (FWL needs a contiguous 128-col weight load)
            rhs_bufs = [
                singles.tile(
                    [128, NCLS, 2 * D], BF16, name=f"rhsbuf{j}", tag=f"rhsbuf{j}"
                )
                for j in range(NRHS)
            ]
            cnt_sb = singles.tile([VIEW_P, NBLK], F32, tag="cnt_sb")

            xb_bufs = [
                singles.tile([128, BLK], F32, name=f"xb{j}", tag=f"xb{j}")
                for j in range(NXB)
            ]
            mi_bufs = [
                singles.tile([VIEW_P, BLK], I32, name=f"mi{j}", tag=f"mi{j}")
                for j in range(NMI)
            ]
            mf_bufs = [
                singles.tile([VIEW_P, BLK], BF16, name=f"mf{j}", tag=f"mf{j}")
                for j in range(2)
            ]
            # mt written once per block, read only during that block's matmuls
            mt_bufs = [
                singles.tile([VIEW_P, BLK], BF16, name=f"mt{j}", tag=f"mt{j}")
                for j in range(3)
            ]
            ps4_bufs = [
                psingles.tile([128, 8, 128], BF16, name=f"ps4{j}", tag=f"ps4{j}")
                for j in range(2)
            ]
            # One full 2 KiB PSUM bank: 8 slices [d, 2q+t, 0:M] (t=0 -> m@x,
            # t=1 -> m@x^2); cols M..64 of each slice are dead padding.
            ps8 = psingles.tile([128, 8, 64], F32, tag="ps8")

            n_mm = NBLK * 4 * NCLS * 2

            def mask_dma(b):
                nc.sync.dma_start(
                    out=mi_bufs[b % NMI], in_=mask[:, b * BLK:(b + 1) * BLK]
                )

            def mask_stt(b):
                # whole-block int32->bf16 convert + free-dim count, one DVE op
                # (measured: DVE STT 4.4 us/block vs 5.6 us for an ACT copy)
                nc.vector.scalar_tensor_tensor(
                    out=mf_bufs[b % 2],
                    in0=mi_bufs[b % NMI], scalar=0.0, in1=mi_bufs[b % NMI],
                    op0=mybir.AluOpType.add,
                    op1=mybir.AluOpType.bypass,
                    accum_out=cnt_sb[:, b:b + 1],
                )

            def transposes(b, h0, h1):
                # Transpose the mask block through the PE with stride-32 free
                # APs: transpose c yields, for every quarter q, the moving-
                # operand mask columns of point class {q*16384 + b*4096 +
                # 32p + c}.
                mf = mf_bufs[b % 2]
                mt = mt_bufs[b % 3]
                # f' = 32p + c: class c picks stride-32 free elems
                mfv = mf.rearrange("r (p c) -> r c p", c=NCLS)
                for h in range(h0, h1):
                    ps4 = ps4_bufs[h % 2]
                    for tt in range(8):
                        c = h * 8 + tt
                        nc.tensor.transpose(ps4[:, tt, :], mfv[:, c, :], ident)
                    nc.scalar.copy(
                        mt[:, h * 1024:(h + 1) * 1024],
                        ps4.rearrange("p t d -> p (t d)"),
                    )

            def x_chunk(jx, xb, rhs, c0, c1):
                # Per-chunk stages for classes [c0, c1): DVE converts the f32
                # landing slice to the bf16 x half (2-port 2x mode -- safe,
                # nothing here uses the GpSimd shared port); the square into
                # the x^2 half runs on ACT mid-stream (it has slack there and
                # keeping DVE short keeps the DMA-pacing conv chain tight),
                # but on DVE for the last block, where ACT's ~0.9 ns/elem
                # squares would otherwise serialize the whole tail.
                xbv = xb.rearrange("p (s d) -> p s d", s=NCLS)
                nc.vector.tensor_scalar_mul(
                    rhs[:, c0:c1, 0:D], xbv[:, c0:c1, :], 1.0
                )
                if jx < NT - 4:
                    nc.scalar.square(
                        rhs[:, c0:c1, D:2 * D], rhs[:, c0:c1, 0:D]
                    )
                else:
                    nc.vector.tensor_mul(
                        rhs[:, c0:c1, D:2 * D],
                        rhs[:, c0:c1, 0:D],
                        rhs[:, c0:c1, 0:D],
                    )

            k = 0
            jx = 0
            mask_dma(0)
            mask_stt(0)
            transposes(0, 0, NCLS // 8)
            for b in range(NBLK):
                mtv = mt_bufs[b % 3].rearrange(
                    "p (c m q) -> p c q m", c=NCLS, m=M, q=4
                )
                for q in range(4):
                    if b == 0 and q >= 1:
                        mask_dma(q)       # all mask triggers early on Sync
                    j = q * NBLK + b       # x tile covering this block+quarter
                    xb = xb_bufs[jx % NXB]
                    rhs = rhs_bufs[jx % NRHS]
                    last_tile = jx == NT - 1
                    jx += 1
                    nchunk = TAILC if last_tile else 1
                    ccls = NCLS // nchunk
                    for ch in range(nchunk):
                        c0, c1 = ch * ccls, (ch + 1) * ccls
                        # two sub-transfers per chunk: a single 2 MB DMA only
                        # sustains ~370 B/ns (per-descriptor gaps); keeping two
                        # transfers in flight restores the ~430 B/ns rate
                        cm = (c0 + c1) // 2
                        nc.sync.dma_start(
                            out=xb[:, c0 * D:cm * D],
                            in_=xt[j, :, c0 * D:cm * D],
                        )
                        nc.sync.dma_start(
                            out=xb[:, cm * D:c1 * D],
                            in_=xt[j, :, cm * D:c1 * D],
                        )
                        x_chunk(jx - 1, xb, rhs, c0, c1)
                        for c in range(c0, c1):
                            for t in range(2):
                                nc.tensor.matmul(
                                    ps8[:, 2 * q + t, 0:M],
                                    lhsT=rhs[:, c, t * D:(t + 1) * D],
                                    rhs=mtv[:, c, q, :],
                                    start=(k == 0),
                                    stop=(k == n_mm - 1),
                                    skip_group_check=True,
                                )
                                k += 1
                    # Software pipeline: next block's mask convert/transpose
                    # interleaves with this block's matmul stream.
                    if b + 1 < NBLK:
                        if q == 1:
                            mask_stt(b + 1)
                        if q >= 2:
                            h0 = (q - 2) * 2
                            transposes(b + 1, h0, h0 + 2)

            outs = singles.tile([128, OUTW], F32, tag="outs")
            nc.vector.tensor_copy(
                outs.rearrange("p (i m) -> p i m", i=8), ps8[:, :, 0:M]
            )
            nc.sync.dma_start(out=out[:, :], in_=outs)
            nc.scalar.dma_start(out=cnts[:, :], in_=cnt_sb)
    # Split multi-wait instructions into EventSemaphore + instruction to
    # satisfy the TRN2 1-wait-per-instruction codegen limit.
    bass_rust.generate_event_semaphores(nc)
    return nc


def _get_nc():
    if "nc" not in _NC_CACHE:
        _NC_CACHE["nc"] = _build_nc()
    return _NC_CACHE["nc"]


def kernel(net_out, pt_offset, mask_embs, mask_pts, logit_scale):
    global LAST
    net_out = np.ascontiguousarray(np.asarray(net_out, dtype=np.float32))
    mask_pts = np.ascontiguousarray(np.asarray(mask_pts, dtype=np.int32))
    mask_embs = np.asarray(mask_embs, dtype=np.float32)

    nc = _get_nc()
    in_maps = [
        {
            "x": net_out[o * P:(o + 1) * P],
            "mask": mask_pts[o].reshape(VIEW_P, VIEW_F),
        }
        for o in range(N_CORES)
    ]
    trace = os.environ.get("KBENCH_TRACE", "0") == "1"
    res = run_bass_kernel_spmd(nc, in_maps, list(range(N_CORES)), trace=trace)
    LAST = res

    # out[d, 2q+t, m]: t=0 -> sum_p x*mask, t=1 -> sum_p x^2*mask (quarter q)
    accs = np.stack([np.asarray(res.results[o]["out"]) for o in range(N_CORES)])
    accs = accs.reshape(N_CORES, D, 4, 2, M).astype(np.float64)
    mx = accs[:, :, :, 0, :].sum(2).transpose(0, 2, 1)   # [8, 32, 128]
    t1 = accs[:, :, :, 1, :].sum((1, 2))                 # [8, 32]
    # cnt[m] = sum over quarters q and blocks of the per-row partials
    cnts = np.stack([np.asarray(res.results[o]["cnts"]) for o in range(N_CORES)])
    cnt = cnts.sum(-1).reshape(N_CORES, M, 4).sum(-1)  # [8, 32]

    emb = np.nan_to_num(
        mask_embs.reshape(N_OBJ, M, D).astype(np.float64),
        nan=0.0, posinf=0.0, neginf=0.0,
    )
    t2 = cnt * (emb * emb).sum(-1)
    t3 = 2.0 * (emb * mx).sum(-1)
    sum_sq = (t1 + t2 - t3).sum()
    total = cnt.sum()
    val = sum_sq / (D * total) if total > 0 else 0.0
    return np.float32(val)
